# revision 21
# baseline (speedup 1.0000x reference)
"""BinaryConnectNet forward pass on 8 Trainium2 NeuronCores (data parallel).

Batch 512 -> 64 per core; binarized weight signs baked host-side and
replicated; shift-BN global batch statistics gathered across the 8 cores.

Per-core design (v3):
  conv1: host im2col paired-image layout xcol [54, 32*1024] f32r; K=54
    block-diag matmuls, 4 stride-2 pool phases x 2 image-pairs per
    iteration -> PSUM; maxpool as a max-tree (ACT copy + DVE TT/TT/STT,
    S1 via STT accum_out, S2 via ACT Square accum_out). Clip never binds
    in block1 (|conv1| << 127) so only the max(-128) side is kept.
  blocks 2-4: depthwise 3x3 (+folded residual) as 9-tap block-diag PE
    matmuls over padded images; 1x1 convs as PE matmuls; clip as DVE
    STT (max with -128, min with a 127-constant tensor) with S1
    accum_out; S2 via ACT Square accum_out (block3 alternates chunks to
    DVE tensor_tensor_reduce to balance engines). PSUM->SBUF staging
    copies split between ACT and DVE. c3 streams through DRAM in bf16;
    dw4 staging and c4/h4 kept bf16.
  BN stats: accum columns reduced on-chip, bh-halves folded via a tiny
    f32 PE matmul, cross-core combine via AllGather + local reduce (one
    collective per BN layer, both channel groups packed in one payload),
    AP2 shift via Ln -> *(-1/(2 ln2)) -> round(int cast) -> Exp,
    vectorized over both groups for BN3/BN4.
  FC: weights preloaded; BN4 applied per-group in contiguous slices so
    group 1's apply overlaps group 0's 256 accumulating bf16 matmuls.
"""
import os
import numpy as np
import ml_dtypes

import concourse.bass as bass
import concourse.bacc as bacc
import concourse.tile as tile
import concourse.mybir as mybir
from concourse import bass_utils

N_CORES = 8
B_CORE = 64
LN2 = float(np.log(2.0))
EPS = 1e-5
F32 = mybir.dt.float32
F32R = mybir.dt.float32r
BF16 = mybir.dt.bfloat16
I32 = mybir.dt.int32
U32 = mybir.dt.uint32
AO = mybir.AluOpType
AF = mybir.ActivationFunctionType
AX = mybir.AxisListType

_CACHE = {}


# ----------------------------------------------------------------- host prep

def _host_prep(x, w1, w21, w31, w41, w22, w32, w42, wfc):
    sgn = lambda w: np.where(np.asarray(w) >= 0, 1.0, -1.0).astype(np.float32)

    xp = np.pad(np.asarray(x, np.float32), ((0, 0), (0, 0), (1, 1), (1, 1)))
    cols = []
    for ci in range(3):
        for ky in range(3):
            for kx in range(3):
                cols.append(xp[:, ci, ky:ky + 32, kx:kx + 32])
    xcol = np.stack(cols, 0).reshape(27, 512, 1024)
    # paired-image layout: [core][54, bl, 1024]; rows 0:27 img bl,
    # rows 27:54 img bl+32 (within the core's 64-image slice)
    xcol2 = np.zeros((N_CORES, 54, 32, 1024), np.float32)
    for c in range(N_CORES):
        sl = xcol[:, c * 64:(c + 1) * 64]
        xcol2[c, 0:27] = sl[:, 0:32]
        xcol2[c, 27:54] = sl[:, 32:64]
    xcol2 = xcol2.reshape(N_CORES, 54, 32 * 1024)

    w1t = sgn(w1).reshape(64, 27).T                       # [27, 64]
    w1t2 = np.zeros((54, 128), np.float32)
    w1t2[0:27, 0:64] = w1t
    w1t2[27:54, 64:128] = w1t

    def diag_pack(wdw, nch):
        s = sgn(wdw).reshape(nch, 9).copy()
        s[:, 4] += 1.0  # fold residual: t = h + dw(h)
        groups = []
        if nch == 64:
            d = np.zeros((128, 9, 128), np.float32)
            for p in range(128):
                d[p, :, p] = s[p % 64]
            groups.append(d.reshape(128, 9 * 128))
        else:
            for g in range(nch // 128):
                d = np.zeros((128, 9, 128), np.float32)
                for p in range(128):
                    d[p, :, p] = s[g * 128 + p]
                groups.append(d.reshape(128, 9 * 128))
        return np.stack(groups)

    d2 = diag_pack(w21, 64)[0]
    d3 = diag_pack(w31, 64)[0]
    d4 = diag_pack(w41, 256)                              # [2, 128, 1152]

    w22s = np.ascontiguousarray(sgn(w22)[:, :, 0, 0].T)   # [64, 64]
    w22t = np.zeros((128, 128), np.float32)               # block-diag (bh)
    w22t[0:64, 0:64] = w22s
    w22t[64:128, 64:128] = w22s
    w32t = np.ascontiguousarray(sgn(w32)[:, :, 0, 0].T)   # [64, 256]
    w32t = np.concatenate([w32t, w32t], 0)                # [128, 256]
    w42t = np.ascontiguousarray(
        sgn(w42)[:, :, 0, 0].T).reshape(2, 128, 256).astype(
            ml_dtypes.bfloat16)                           # [kg][ci, 256co]

    wf = sgn(wfc).reshape(10, 256, 256)                   # [o, c, pix]
    wfct = np.ascontiguousarray(
        wf.transpose(1, 2, 0)).reshape(2, 128, 2560).astype(
            ml_dtypes.bfloat16)                           # [kg][ci, pix*10+o]

    dwv4 = sgn(w41).reshape(256, 9).reshape(2, 128, 9)
    dwv4 = np.ascontiguousarray(dwv4.transpose(1, 0, 2))  # [128, 2, 9]

    foldf = np.zeros((128, 64), np.float32)               # bh-pair fold
    for k in range(128):
        foldf[k, k % 64] = 1.0
    bcastb = np.zeros((64, 128), np.float32)              # 64 -> 128 bcast
    for m in range(128):
        bcastb[m % 64, m] = 1.0

    return (xcol2, w1t2, d2, d3, d4, w22t, w32t, w42t, wfct, foldf,
            bcastb, dwv4)


# ------------------------------------------------------------ device pieces

def _emit_dw(nc, ps, hpad_view, diag_sb, nb_img, psum_tag, taps=None):
    """Depthwise(+identity) over padded images [128, nb_img, 18, 18].
    Returns psum tile [128, nb_img, 16, 16] (full rectangles only)."""
    p = ps.tile([128, nb_img, 16, 16], F32, tag=psum_tag)
    order = taps or [4, 0, 1, 2, 3, 5, 6, 7, 8]
    for i, t in enumerate(order):
        dy, dx = t // 3, t % 3
        nc.tensor.matmul(
            p[:], diag_sb[:, t, :],
            hpad_view[:, :, dy:dy + 16, dx:dx + 16],
            start=(i == 0), stop=(i == len(order) - 1))
    return p


def _emit_ab(nc, sm, s1_ap, s2_ap, P, W, n_tot, gamma, beta, tag):
    """(S1, S2) global sums [P,W] each -> (a, b) [P,W] BN coefficients."""
    mu = sm.tile([P, W], F32, tag=tag + "mu")
    nc.vector.tensor_scalar(mu[:], s1_ap, 1.0 / n_tot, None, op0=AO.mult)
    exx = sm.tile([P, W], F32, tag=tag + "ex")
    nc.vector.tensor_scalar(exx[:], s2_ap, 1.0 / n_tot, None, op0=AO.mult)
    musq = sm.tile([P, W], F32, tag=tag + "m2")
    nc.vector.tensor_tensor(musq[:], mu[:], mu[:], op=AO.mult)
    var = sm.tile([P, W], F32, tag=tag + "va")
    nc.vector.tensor_tensor(var[:], exx[:], musq[:], op=AO.subtract)
    eps_t = sm.tile([P, 1], F32, tag=tag + "ep")
    nc.vector.memset(eps_t[:], EPS)
    lg = sm.tile([P, W], F32, tag=tag + "lg")
    nc.scalar.activation(lg[:], var[:], AF.Ln, bias=eps_t[:], scale=1.0)
    t = sm.tile([P, W], F32, tag=tag + "t")
    nc.vector.tensor_scalar(t[:], lg[:], -0.5 / LN2, None, op0=AO.mult)
    ti = sm.tile([P, W], I32, tag=tag + "ti")
    nc.vector.tensor_copy(ti[:], t[:])        # fp32->int32 rounds to nearest
    tf = sm.tile([P, W], F32, tag=tag + "tf")
    nc.vector.tensor_copy(tf[:], ti[:])
    zb = sm.tile([P, 1], F32, tag=tag + "zb")
    nc.vector.memset(zb[:], 0.0)
    sh = sm.tile([P, W], F32, tag=tag + "sh")
    nc.scalar.activation(sh[:], tf[:], AF.Exp, bias=zb[:], scale=LN2)
    a = sm.tile([P, W], F32, tag=tag + "a")
    nc.vector.tensor_tensor(a[:], sh[:], gamma, op=AO.mult)
    amu = sm.tile([P, W], F32, tag=tag + "am")
    nc.vector.tensor_tensor(amu[:], a[:], mu[:], op=AO.mult)
    b = sm.tile([P, W], F32, tag=tag + "b")
    nc.vector.tensor_tensor(b[:], beta, amu[:], op=AO.subtract)
    return a, b


def _allgather_sum(nc, sm, dram, src_ap, P, W, tag):
    """AllGather [P, W] across 8 cores, then local reduce -> [P, W] sums."""
    ag_in = dram.tile([P, W], F32, tag=tag + "i")
    ag_out = dram.tile([N_CORES, P, W], F32, tag=tag + "o")
    nc.gpsimd.dma_start(out=ag_in[:], in_=src_ap)
    nc.gpsimd.collective_compute(
        "AllGather", AO.bypass, replica_groups=[list(range(N_CORES))],
        ins=[ag_in.opt()], outs=[ag_out.opt()])
    g8 = sm.tile([P, W, N_CORES], F32, tag=tag + "g")
    nc.sync.dma_start(out=g8, in_=ag_out[:].rearrange("r p w -> p w r"))
    sg = sm.tile([P, W], F32, tag=tag + "s")
    nc.vector.tensor_reduce(sg[:], g8[:], axis=AX.X, op=AO.add)
    return sg


def _border_memset(nc, t):
    """Zero only the 1-px border of padded [128, nimg, 18, 18] tiles."""
    nc.vector.memset(t[:, :, 0:18:17, :].bitcast(U32), 0)
    nc.vector.memset(t[:, :, 1:17, 0:18:17].bitcast(U32), 0)


# ------------------------------------------------------------- device build

def build(debug=False):
    nc = bacc.Bacc("TRN2", target_bir_lowering=False, debug=False,
                   num_devices=N_CORES)
    din = {}

    def dd(name, shape, dtype=F32R):
        din[name] = nc.dram_tensor(name, list(shape), dtype,
                                   kind="ExternalInput")

    dd("xcol", [54, 32 * 1024])
    dd("w1t2", [54, 128])
    dd("d2", [128, 9 * 128])
    dd("d3", [128, 9 * 128])
    dd("d4", [2, 128, 9 * 128])
    dd("w22t", [128, 128])
    dd("w32t", [128, 256])
    dd("w42t", [2, 128, 256], BF16)
    dd("wfct", [2, 128, 2560], BF16)
    dd("gb", [128, 10], F32)
    dd("gb2", [128, 4], F32)
    dd("foldf", [128, 64], F32)
    dd("dwv4", [128, 2, 9], F32)
    dd("bcastb", [64, 128], F32)
    out_d = nc.dram_tensor("out", [10, B_CORE], F32, kind="ExternalOutput")

    dbg = {}
    if debug:
        for name, shape, dt in [
                ("c1", [128, 32, 18, 18], F32), ("sg1", [64, 2], F32),
                ("ab1", [128, 2], F32), ("h1", [128, 32, 18, 18], F32),
                ("pk2", [128, 2], F32), ("sg2", [64, 2], F32),
                ("ab2", [128, 2], F32),
                ("c2", [128, 32, 18, 18], F32), ("h2", [128, 32, 18, 18], F32),
                ("c3", [2, 128, 16384], BF16),
                ("c4", [2, 128, 16384], BF16), ("h4", [2, 128, 16384], BF16)]:
            dbg[name] = nc.dram_tensor("dbg_" + name, shape, dt,
                                       kind="ExternalOutput")

    with tile.TileContext(nc) as tc:
        with tc.tile_pool(name="wts", bufs=1) as wts, \
             tc.tile_pool(name="sb", bufs=1) as sb, \
             tc.tile_pool(name="sm", bufs=1) as sm, \
             tc.tile_pool(name="xin", bufs=2) as xin, \
             tc.tile_pool(name="cho", bufs=3) as cho, \
             tc.tile_pool(name="ps", bufs=2, space="PSUM") as ps, \
             tc.tile_pool(name="psA", bufs=2, space="PSUM") as psA, \
             tc.tile_pool(name="dram", bufs=1, space="DRAM") as dram:
            _body(nc, tc, wts, sb, sm, xin, cho, ps, psA, dram,
                  din, out_d, dbg)
    nc.compile()
    return nc


def _body(nc, tc, wts, sb, sm, xin, cho, ps, psA, dram,
          din, out_d, dbg):
    # ---------- all weights upfront (HWDGE is idle early; SBUF affordable)
    def wload(name, shape, dtype=F32R):
        t = wts.tile(list(shape), dtype, tag=name)
        nc.sync.dma_start(out=t, in_=din[name].ap())
        return t

    w1t2 = wload("w1t2", [54, 128])
    d2 = wload("d2", [128, 9, 128])
    d3 = wload("d3", [128, 9, 128])
    d4 = wts.tile([128, 2, 9, 128], F32R, tag="d4")
    for g in range(2):
        nc.sync.dma_start(out=d4[:, g], in_=din["d4"].ap()[g])
    w22t = wload("w22t", [128, 128])
    w32t = wload("w32t", [128, 256])
    w42t = wts.tile([128, 2, 256], BF16, tag="w42t")
    for g in range(2):
        nc.sync.dma_start(out=w42t[:, g], in_=din["w42t"].ap()[g])
    wfct = wts.tile([128, 2, 2560], BF16, tag="wfct")
    for g in range(2):
        nc.sync.dma_start(out=wfct[:, g], in_=din["wfct"].ap()[g])
    gb = wload("gb", [128, 10], F32)
    gb2 = wload("gb2", [128, 4], F32)
    foldf = wload("foldf", [128, 64], F32)
    dwv4 = wload("dwv4", [128, 2, 9], F32)
    bcastb = wload("bcastb", [64, 128], F32)

    sqa = sm.tile([128, 512], F32, tag="sqa")   # ACT Square dump
    sqd = sm.tile([128, 512], F32, tag="sqd")   # DVE TTR dump
    c127 = wts.tile([128, 512], F32, tag="c127")
    nc.vector.memset(c127[:], 127.0)

    def bn_fold_gather(pk, gamma64, beta64, tag):
        """pk [128,2]=(S1,S2) per (bh,ch) -> ab128 [128,2] applied coeffs."""
        pm = ps.tile([128, 512], F32, tag="pu")
        pf = pm[0:64, 0:2]
        nc.tensor.matmul(pf, foldf[:], pk[:], start=True, stop=True)
        sA = sm.tile([64, 2], F32, tag=tag + "sA")
        nc.vector.tensor_copy(sA[:], pf)
        sg = _allgather_sum(nc, sm, dram, sA[:], 64, 2, tag + "ag")
        if dbg and tag == "bn1":
            nc.sync.dma_start(out=dbg["sg1"].ap(), in_=sg[:])
        if dbg and tag == "bn2":
            nc.sync.dma_start(out=dbg["pk2"].ap(), in_=pk[:])
            nc.sync.dma_start(out=dbg["sg2"].ap(), in_=sg[:])
        a64, b64 = _emit_ab(nc, sm, sg[:, 0:1], sg[:, 1:2], 64, 1, 131072,
                            gamma64, beta64, tag)
        ab2 = sm.tile([64, 2], F32, tag=tag + "ab2")
        nc.vector.tensor_copy(ab2[:, 0:1], a64[:])
        nc.vector.tensor_copy(ab2[:, 1:2], b64[:])
        pm2 = ps.tile([128, 512], F32, tag="pu")
        pb = pm2[:, 0:2]
        nc.tensor.matmul(pb, bcastb[:], ab2[:], start=True, stop=True)
        ab128 = sm.tile([128, 2], F32, tag=tag + "abb")
        nc.vector.tensor_copy(ab128[:], pb)
        if dbg and tag == "bn1":
            nc.sync.dma_start(out=dbg["ab1"].ap(), in_=ab128[:])
        if dbg and tag == "bn2":
            nc.sync.dma_start(out=dbg["ab2"].ap(), in_=ab128[:])
        return ab128

    # ---------- stage A: conv1 (paired K=54) + 4-phase maxpool -> c1 padded
    c1 = sb.tile([128, 32, 18, 18], F32R, tag="chainA")
    _border_memset(nc, c1)
    c2 = sb.tile([128, 32, 18, 18], F32R, tag="chainB")
    _border_memset(nc, c2)
    h3c = []
    for g in range(2):
        for s in range(2):
            t = sb.tile([128, 2, 18, 18], F32R, tag=f"h3c{g}{s}")
            _border_memset(nc, t)
            h3c.append(t)

    s1a = sm.tile([128, 32], F32, tag="s1a")
    s2a = sm.tile([128, 16], F32, tag="s2a")
    for it in range(16):
        bl = it * 2
        xc = xin.tile([54, 2048], F32R, tag="xc")
        nc.sync.dma_start(
            out=xc, in_=din["xcol"].ap()[:, bl * 1024:(bl + 2) * 1024])
        xv = xc[:].rearrange("k (q y a x b) -> k a b q y x",
                             q=2, y=16, a=2, x=16)
        pA0 = psA.tile([128, 2, 512], F32, tag="pA")
        pA1 = psA.tile([128, 2, 512], F32, tag="pA")
        for i, (dy, dx) in enumerate(((0, 0), (0, 1), (1, 0), (1, 1))):
            dstp = pA0[:, i] if i < 2 else pA1[:, i - 2]
            nc.tensor.matmul(
                dstp.rearrange("p (q y x) -> p q y x", q=2, y=16),
                w1t2[:], xv[:, dy, dx], start=True, stop=True)
        u0 = xin.tile([128, 512], F32, tag="u0")
        nc.scalar.copy(u0[:], pA0[:, 0])
        mA = xin.tile([128, 512], F32, tag="mA")
        nc.vector.tensor_tensor(mA[:], pA0[:, 1], u0[:], op=AO.max)
        mB = xin.tile([128, 512], F32, tag="mB")
        nc.vector.tensor_tensor(mB[:], pA1[:, 0], mA[:], op=AO.max)
        for q in range(2):
            nc.vector.scalar_tensor_tensor(
                c1[:, bl + q, 1:17, 1:17],
                pA1[:, 1, q * 256:(q + 1) * 256]
                .rearrange("p (y x) -> p y x", y=16),
                -128.0,
                mB[:, q * 256:(q + 1) * 256]
                .rearrange("p (y x) -> p y x", y=16),
                op0=AO.max, op1=AO.max,
                accum_out=s1a[:, bl + q:bl + q + 1])
        nc.scalar.activation(
            sqa[:].rearrange("p (a y x) -> p a y x", a=2, y=16),
            c1[:, bl:bl + 2, 1:17, 1:17], AF.Square,
            accum_out=s2a[:, it:it + 1])
    if dbg:
        nc.sync.dma_start(out=dbg["c1"].ap(), in_=c1[:].bitcast(F32))

    # ---------- BN1 (fold bh, gather, AP2) + apply in place
    pk1 = sm.tile([128, 2], F32, tag="pk1")
    nc.vector.tensor_reduce(pk1[:, 0:1], s1a[:], axis=AX.X, op=AO.add)
    nc.vector.tensor_reduce(pk1[:, 1:2], s2a[:], axis=AX.X, op=AO.add)
    ab1 = bn_fold_gather(pk1, gb[0:64, 0:1], gb[0:64, 1:2], "bn1")
    for k in range(8):
        iv = c1[:, 4 * k:4 * k + 4, 1:17, 1:17]
        nc.scalar.activation(iv, iv, AF.Relu, bias=ab1[:, 1:2],
                             scale=ab1[:, 0:1])
    h1 = c1
    if dbg:
        nc.sync.dma_start(out=dbg["h1"].ap(), in_=c1[:].bitcast(F32))

    # ---------- block2: dw2 + 1x1(64->64) -> c2 padded ; BN2 in place
    s1b = sm.tile([128, 32], F32, tag="s1b")
    s2b = sm.tile([128, 16], F32, tag="s2b")
    for b0 in range(0, 32, 2):
        p = _emit_dw(nc, ps, h1[:, b0:b0 + 2], d2, 2, "pdw")
        t2 = cho.tile([128, 2, 16, 16], F32R, tag="tdwA")
        nc.vector.tensor_copy(t2[:], p[:])
        ci = b0 // 2
        pu = ps.tile([128, 512], F32, tag="pu")
        nc.tensor.matmul(
            pu[:], w22t[:], t2[:].rearrange("p a b c -> p (a b c)"),
            start=True, stop=True)
        puv = pu[:].rearrange("p (a b c) -> p a b c", a=2, b=16)
        c127v = c127[:, 0:256].rearrange("p (y x) -> p y x", y=16)
        for si in range(2):
            nc.vector.scalar_tensor_tensor(
                c2[:, b0 + si, 1:17, 1:17], puv[:, si], -128.0, c127v,
                op0=AO.max, op1=AO.min,
                accum_out=s1b[:, 2 * ci + si:2 * ci + si + 1])
        iv = c2[:, b0:b0 + 2, 1:17, 1:17]
        nc.scalar.activation(sqa[:].rearrange("p (a b c) -> p a b c",
                                              a=2, b=16),
                             iv, AF.Square, accum_out=s2b[:, ci:ci + 1])
    if dbg:
        nc.sync.dma_start(out=dbg["c2"].ap(), in_=c2[:].bitcast(F32))

    pk2 = sm.tile([128, 2], F32, tag="pk2")
    nc.vector.tensor_reduce(pk2[:, 0:1], s1b[:], axis=AX.X, op=AO.add)
    nc.vector.tensor_reduce(pk2[:, 1:2], s2b[:], axis=AX.X, op=AO.add)
    ab2c = bn_fold_gather(pk2, gb[0:64, 2:3], gb[0:64, 3:4], "bn2")
    for k in range(8):
        iv = c2[:, 4 * k:4 * k + 4, 1:17, 1:17]
        nc.scalar.activation(iv, iv, AF.Relu, bias=ab2c[:, 1:2],
                             scale=ab2c[:, 0:1])
    h2 = c2
    if dbg:
        nc.sync.dma_start(out=dbg["h2"].ap(), in_=c2[:].bitcast(F32))

    # ---------- block3: dw3 + 1x1(64->256) -> c3 DRAM bf16 (both groups)
    c3_dram = dram.tile([2, 128, 16384], BF16, tag="c3d")
    s13 = sm.tile([128, 2, 32], F32, tag="s13")
    s23 = sm.tile([128, 2, 32], F32, tag="s23")
    for b0 in range(0, 32, 2):
        p = _emit_dw(nc, ps, h2[:, b0:b0 + 2], d3, 2, "pdw")
        t3 = cho.tile([128, 2, 16, 16], F32R, tag="tdwA")
        nc.scalar.copy(t3[:], p[:])
        ci = b0 // 2
        for bh in range(2):
            b_abs = bh * 32 + b0
            for g in range(2):
                pu = ps.tile([128, 512], F32, tag="pu")
                nc.tensor.matmul(
                    pu[:], w32t[bh * 64:(bh + 1) * 64,
                                g * 128:(g + 1) * 128],
                    t3[bh * 64:(bh + 1) * 64]
                    .rearrange("p a b c -> p (a b c)"),
                    start=True, stop=True)
                cc = cho.tile([128, 512], BF16, tag="ccs")
                kcol = bh * 16 + ci
                nc.vector.scalar_tensor_tensor(
                    cc[:], pu[:], -128.0, c127[:], op0=AO.max, op1=AO.min,
                    accum_out=s13[:, g, kcol:kcol + 1])
                if bh == 1 and g == 1:
                    nc.vector.scalar_tensor_tensor(
                        sqd[:], cc[:], 1.0, cc[:],
                        op0=AO.mult, op1=AO.mult,
                        accum_out=s23[:, g, kcol:kcol + 1])
                else:
                    nc.scalar.activation(
                        sqa[:], cc[:], AF.Square,
                        accum_out=s23[:, g, kcol:kcol + 1])
                nc.sync.dma_start(
                    out=c3_dram[g, :, b_abs * 256:(b_abs + 2) * 256],
                    in_=cc[:])
    if dbg:
        for g in range(2):
            nc.sync.dma_start(out=dbg["c3"].ap()[g], in_=c3_dram[g])

    # ---------- BN3: pack both groups (S1g0,S1g1,S2g0,S2g1), one gather
    pk3 = sm.tile([128, 4], F32, tag="pk3")
    for g in range(2):
        nc.vector.tensor_reduce(pk3[:, g:g + 1], s13[:, g],
                                axis=AX.X, op=AO.add)
        nc.vector.tensor_reduce(pk3[:, 2 + g:3 + g], s23[:, g],
                                axis=AX.X, op=AO.add)
    sg3 = _allgather_sum(nc, sm, dram, pk3[:], 128, 4, "bn3ag")
    a3, b3 = _emit_ab(nc, sm, sg3[:, 0:2], sg3[:, 2:4], 128, 2, 131072,
                      gb[:, 4:6], gb[:, 6:8], "bn3")

    # ---------- block4: stream c3 bf16, BN3 on the fly, dw4, 1x1 -> c4 bf16
    c4_g0 = sb.tile([128, 64, 16, 16], BF16, tag="chainA")
    c4_g1 = sb.tile([128, 64, 16, 16], BF16, tag="chainB")
    c4 = [c4_g0, c4_g1]
    s14 = sm.tile([128, 2, 32], F32, tag="s14")
    s24 = sm.tile([128, 2, 32], F32, tag="s24")
    for b0 in range(0, 64, 2):
        ci = b0 // 2
        t4 = []
        for g in range(2):
            c3c = cho.tile([128, 512], BF16, tag="c3c")
            nc.sync.dma_start(out=c3c,
                              in_=c3_dram[g, :, b0 * 256:(b0 + 2) * 256])
            hp = h3c[g * 2 + (ci % 2)]
            nc.scalar.activation(
                hp[:, :, 1:17, 1:17],
                c3c[:].rearrange("p (a b c) -> p a b c", a=2, b=16),
                AF.Relu, bias=b3[:, g:g + 1], scale=a3[:, g:g + 1])
            p = _emit_dw(nc, ps, hp[:], d4[:, g], 2, "pdw",
                         taps=[4, 0, 1, 2, 3, 5, 6])
            tg = cho.tile([128, 2, 16, 16], BF16, tag=f"tdw{'AB'[g]}")
            nc.scalar.copy(tg[:], p[:])
            for t in (7, 8):
                dy, dx = t // 3, t % 3
                for si in range(2):
                    nc.vector.scalar_tensor_tensor(
                        tg[:, si], hp[:, si, dy:dy + 16, dx:dx + 16],
                        dwv4[:, g, t:t + 1], tg[:, si],
                        op0=AO.mult, op1=AO.add)
            t4.append(tg)
        for mg in range(2):
            pu = ps.tile([128, 512], F32, tag="pu")
            for kg in range(2):
                nc.tensor.matmul(
                    pu[:], w42t[:, kg, mg * 128:(mg + 1) * 128],
                    t4[kg][:].rearrange("p a b c -> p (a b c)"),
                    start=(kg == 0), stop=(kg == 1))
            dst = c4[mg][:, b0:b0 + 2].rearrange("p a b c -> p (a b c)")
            nc.vector.scalar_tensor_tensor(
                dst, pu[:], -128.0, c127[:], op0=AO.max, op1=AO.min,
                accum_out=s14[:, mg, ci:ci + 1])
            nc.scalar.activation(sqa[:], dst, AF.Square,
                                 accum_out=s24[:, mg, ci:ci + 1])
    if dbg:
        for g in range(2):
            nc.gpsimd.dma_start(
                out=dbg["c4"].ap()[g],
                in_=c4[g][:].rearrange("p a b c -> p (a b c)"))

    # ---------- BN4: one gather; apply per group, overlapped with FC
    pk4 = sm.tile([128, 4], F32, tag="pk4")
    for g in range(2):
        nc.vector.tensor_reduce(pk4[:, g:g + 1], s14[:, g],
                                axis=AX.X, op=AO.add)
        nc.vector.tensor_reduce(pk4[:, 2 + g:3 + g], s24[:, g],
                                axis=AX.X, op=AO.add)
    sg4 = _allgather_sum(nc, sm, dram, pk4[:], 128, 4, "bn4ag")
    a4, b4 = _emit_ab(nc, sm, sg4[:, 0:2], sg4[:, 2:4], 128, 2, 131072,
                      gb2[:, 0:2], gb2[:, 2:4], "bn4")

    # ---------- FC (bf16): 512 accumulating matmuls K=128, M=10, N=64
    pfm = ps.tile([128, 512], F32, tag="pu")
    pf = pfm[0:10, 0:64]
    n_mm = 0
    for kg in range(2):
        for sl in range(4):
            iv = c4[kg][:, sl * 16:(sl + 1) * 16]
            if kg == 0:
                nc.vector.tensor_scalar(iv, iv, a4[:, 0:1], b4[:, 0:1],
                                        op0=AO.mult, op1=AO.add)
                nc.vector.tensor_scalar(iv, iv, 0.0, None, op0=AO.max)
            else:
                nc.scalar.activation(iv, iv, AF.Relu, bias=b4[:, 1:2],
                                     scale=a4[:, 1:2])
        h4v = c4[kg][:].rearrange("p b y x -> p b (y x)")
        if dbg:
            nc.gpsimd.dma_start(
                out=dbg["h4"].ap()[kg],
                in_=c4[kg][:].rearrange("p a b c -> p (a b c)"))
        for pix in range(256):
            n_mm += 1
            nc.tensor.matmul(pf,
                             wfct[:, kg, pix * 10:(pix + 1) * 10],
                             h4v[:, :, pix],
                             start=(n_mm == 1), stop=(n_mm == 512))
    of = sm.tile([10, 64], F32, tag="of")
    nc.vector.tensor_copy(of[:], pf)
    nc.sync.dma_start(out=out_d.ap(), in_=of[:])


# ------------------------------------------------------------------ kernel

def _prep_inputs(x, w1, w21, w22, w31, w32, w41, w42,
                 g1, b1, g2, b2, g3, b3, g4, b4, wfc):
    (xcol2, w1t2, d2, d3, d4, w22t, w32t, w42t, wfct, foldf,
     bcastb, dwv4) = _host_prep(x, w1, w21, w31, w41, w22, w32, w42, wfc)
    f32 = lambda v: np.asarray(v, np.float32)
    g1, b1, g2, b2 = f32(g1), f32(b1), f32(g2), f32(b2)
    g3, b3, g4, b4 = f32(g3), f32(b3), f32(g4), f32(b4)
    gb = np.zeros((128, 10), np.float32)
    gb[:, 0] = np.tile(g1, 2); gb[:, 1] = np.tile(b1, 2)
    gb[:, 2] = np.tile(g2, 2); gb[:, 3] = np.tile(b2, 2)
    gb[:, 4] = g3[:128]; gb[:, 5] = g3[128:]
    gb[:, 6] = b3[:128]; gb[:, 7] = b3[128:]
    gb2 = np.zeros((128, 4), np.float32)
    gb2[:, 0] = g4[:128]; gb2[:, 1] = g4[128:]
    gb2[:, 2] = b4[:128]; gb2[:, 3] = b4[128:]
    in_maps = []
    for c in range(N_CORES):
        in_maps.append({
            "xcol": xcol2[c], "w1t2": w1t2, "d2": d2, "d3": d3, "d4": d4,
            "w22t": w22t, "w32t": w32t, "w42t": w42t, "wfct": wfct,
            "gb": gb, "gb2": gb2, "foldf": foldf, "bcastb": bcastb,
            "dwv4": dwv4,
        })
    return in_maps


def kernel(x, w1, w21, w22, w31, w32, w41, w42,
           g1, b1, g2, b2, g3, b3, g4, b4, wfc, bfc):
    debug = bool(int(os.environ.get("BCK_DEBUG", "0")))
    key = ("nc", debug)
    if key not in _CACHE:
        _CACHE[key] = build(debug=debug)
    nc = _CACHE[key]
    in_maps = _prep_inputs(x, w1, w21, w22, w31, w32, w41, w42,
                           g1, b1, g2, b2, g3, b3, g4, b4, wfc)
    res = bass_utils.run_bass_kernel_spmd(
        nc, in_maps, core_ids=list(range(N_CORES)))
    kernel.last_results = res
    outs = [res.results[c]["out"] for c in range(N_CORES)]
    full = np.concatenate([o.T for o in outs], axis=0)  # [512, 10]
    return (full + np.asarray(bfc, np.float32)[None, :]).astype(np.float32)


# revision 23
# speedup vs baseline: 1.0555x; 1.0555x over previous
"""BinaryConnectNet forward pass on 8 Trainium2 NeuronCores (data parallel).

Batch 512 -> 64 per core; binarized weight signs baked host-side and
replicated; shift-BN global batch statistics gathered across the 8 cores.

Per-core design (v3):
  conv1: host im2col paired-image layout xcol [54, 32*1024] f32r; K=54
    block-diag matmuls, 4 stride-2 pool phases x 2 image-pairs per
    iteration -> PSUM; maxpool as a max-tree (ACT copy + DVE TT/TT/STT,
    S1 via STT accum_out, S2 via ACT Square accum_out). Clip never binds
    in block1 (|conv1| << 127) so only the max(-128) side is kept.
  blocks 2-4: depthwise 3x3 (+folded residual) as 9-tap block-diag PE
    matmuls over padded images; 1x1 convs as PE matmuls; clip as DVE
    STT (max with -128, min with a 127-constant tensor) with S1
    accum_out; S2 via ACT Square accum_out (block3 alternates chunks to
    DVE tensor_tensor_reduce to balance engines). PSUM->SBUF staging
    copies split between ACT and DVE. c3 streams through DRAM in bf16;
    dw4 staging and c4/h4 kept bf16.
  BN stats: accum columns reduced on-chip, bh-halves folded via a tiny
    f32 PE matmul, cross-core combine via AllGather + local reduce (one
    collective per BN layer, both channel groups packed in one payload),
    AP2 shift via Ln -> *(-1/(2 ln2)) -> round(int cast) -> Exp,
    vectorized over both groups for BN3/BN4.
  FC: weights preloaded; BN4 applied per-group in contiguous slices so
    group 1's apply overlaps group 0's 256 accumulating bf16 matmuls.
"""
import os
import numpy as np
import ml_dtypes

import concourse.bass as bass
import concourse.bacc as bacc
import concourse.tile as tile
import concourse.mybir as mybir
from concourse import bass_utils

N_CORES = 8
B_CORE = 64
LN2 = float(np.log(2.0))
EPS = 1e-5
F32 = mybir.dt.float32
F32R = mybir.dt.float32r
BF16 = mybir.dt.bfloat16
I32 = mybir.dt.int32
U32 = mybir.dt.uint32
AO = mybir.AluOpType
AF = mybir.ActivationFunctionType
AX = mybir.AxisListType

_CACHE = {}


# ----------------------------------------------------------------- host prep

def _host_prep(x, w1, w21, w31, w41, w22, w32, w42, wfc):
    sgn = lambda w: np.where(np.asarray(w) >= 0, 1.0, -1.0).astype(np.float32)

    xp = np.pad(np.asarray(x, np.float32), ((0, 0), (0, 0), (1, 1), (1, 1)))
    cols = []
    for ci in range(3):
        for ky in range(3):
            for kx in range(3):
                cols.append(xp[:, ci, ky:ky + 32, kx:kx + 32])
    xcol = np.stack(cols, 0).reshape(27, 512, 1024)
    # paired-image layout: [core][54, bl, 1024]; rows 0:27 img bl,
    # rows 27:54 img bl+32 (within the core's 64-image slice)
    xcol2 = np.zeros((N_CORES, 54, 32, 1024), np.float32)
    for c in range(N_CORES):
        sl = xcol[:, c * 64:(c + 1) * 64]
        xcol2[c, 0:27] = sl[:, 0:32]
        xcol2[c, 27:54] = sl[:, 32:64]
    xcol2 = xcol2.reshape(N_CORES, 54, 32 * 1024)

    w1t = sgn(w1).reshape(64, 27).T                       # [27, 64]
    w1t2 = np.zeros((54, 128), np.float32)
    w1t2[0:27, 0:64] = w1t
    w1t2[27:54, 64:128] = w1t

    def diag_pack(wdw, nch):
        s = sgn(wdw).reshape(nch, 9).copy()
        s[:, 4] += 1.0  # fold residual: t = h + dw(h)
        groups = []
        if nch == 64:
            d = np.zeros((128, 9, 128), np.float32)
            for p in range(128):
                d[p, :, p] = s[p % 64]
            groups.append(d.reshape(128, 9 * 128))
        else:
            for g in range(nch // 128):
                d = np.zeros((128, 9, 128), np.float32)
                for p in range(128):
                    d[p, :, p] = s[g * 128 + p]
                groups.append(d.reshape(128, 9 * 128))
        return np.stack(groups)

    d2 = diag_pack(w21, 64)[0]
    d3 = diag_pack(w31, 64)[0]
    d4 = diag_pack(w41, 256)                              # [2, 128, 1152]

    w22s = np.ascontiguousarray(sgn(w22)[:, :, 0, 0].T)   # [64, 64]
    w22t = np.zeros((128, 128), np.float32)               # block-diag (bh)
    w22t[0:64, 0:64] = w22s
    w22t[64:128, 64:128] = w22s
    w32t = np.ascontiguousarray(sgn(w32)[:, :, 0, 0].T)   # [64, 256]
    w32t = np.concatenate([w32t, w32t], 0)                # [128, 256]
    w42t = np.ascontiguousarray(
        sgn(w42)[:, :, 0, 0].T).reshape(2, 128, 256).astype(
            ml_dtypes.bfloat16)                           # [kg][ci, 256co]

    wf = sgn(wfc).reshape(10, 256, 256)                   # [o, c, pix]
    wfct = np.ascontiguousarray(
        wf.transpose(1, 2, 0)).reshape(2, 128, 2560).astype(
            ml_dtypes.bfloat16)                           # [kg][ci, pix*10+o]

    dwv4 = sgn(w41).reshape(256, 9).reshape(2, 128, 9)
    dwv4 = np.ascontiguousarray(dwv4.transpose(1, 0, 2))  # [128, 2, 9]

    foldf = np.zeros((128, 64), np.float32)               # bh-pair fold
    for k in range(128):
        foldf[k, k % 64] = 1.0
    bcastb = np.zeros((64, 128), np.float32)              # 64 -> 128 bcast
    for m in range(128):
        bcastb[m % 64, m] = 1.0

    return (xcol2, w1t2, d2, d3, d4, w22t, w32t, w42t, wfct, foldf,
            bcastb, dwv4)


# ------------------------------------------------------------ device pieces

def _emit_dw(nc, ps, hpad_view, diag_sb, nb_img, psum_tag, taps=None):
    """Depthwise(+identity) over padded images [128, nb_img, 18, 18].
    Returns psum tile [128, nb_img, 16, 16] (full rectangles only)."""
    p = ps.tile([128, nb_img, 16, 16], F32, tag=psum_tag)
    order = taps or [4, 0, 1, 2, 3, 5, 6, 7, 8]
    for i, t in enumerate(order):
        dy, dx = t // 3, t % 3
        nc.tensor.matmul(
            p[:], diag_sb[:, t, :],
            hpad_view[:, :, dy:dy + 16, dx:dx + 16],
            start=(i == 0), stop=(i == len(order) - 1))
    return p


def _emit_ab(nc, sm, s1_ap, s2_ap, P, W, n_tot, gamma, beta, tag):
    """(S1, S2) global sums [P,W] each -> (a, b) [P,W] BN coefficients."""
    mu = sm.tile([P, W], F32, tag=tag + "mu")
    nc.vector.tensor_scalar(mu[:], s1_ap, 1.0 / n_tot, None, op0=AO.mult)
    exx = sm.tile([P, W], F32, tag=tag + "ex")
    nc.vector.tensor_scalar(exx[:], s2_ap, 1.0 / n_tot, None, op0=AO.mult)
    musq = sm.tile([P, W], F32, tag=tag + "m2")
    nc.vector.tensor_tensor(musq[:], mu[:], mu[:], op=AO.mult)
    var = sm.tile([P, W], F32, tag=tag + "va")
    nc.vector.tensor_tensor(var[:], exx[:], musq[:], op=AO.subtract)
    eps_t = sm.tile([P, 1], F32, tag=tag + "ep")
    nc.vector.memset(eps_t[:], EPS)
    lg = sm.tile([P, W], F32, tag=tag + "lg")
    nc.scalar.activation(lg[:], var[:], AF.Ln, bias=eps_t[:], scale=1.0)
    t = sm.tile([P, W], F32, tag=tag + "t")
    nc.vector.tensor_scalar(t[:], lg[:], -0.5 / LN2, None, op0=AO.mult)
    ti = sm.tile([P, W], I32, tag=tag + "ti")
    nc.vector.tensor_copy(ti[:], t[:])        # fp32->int32 rounds to nearest
    tf = sm.tile([P, W], F32, tag=tag + "tf")
    nc.vector.tensor_copy(tf[:], ti[:])
    zb = sm.tile([P, 1], F32, tag=tag + "zb")
    nc.vector.memset(zb[:], 0.0)
    sh = sm.tile([P, W], F32, tag=tag + "sh")
    nc.scalar.activation(sh[:], tf[:], AF.Exp, bias=zb[:], scale=LN2)
    a = sm.tile([P, W], F32, tag=tag + "a")
    nc.vector.tensor_tensor(a[:], sh[:], gamma, op=AO.mult)
    amu = sm.tile([P, W], F32, tag=tag + "am")
    nc.vector.tensor_tensor(amu[:], a[:], mu[:], op=AO.mult)
    b = sm.tile([P, W], F32, tag=tag + "b")
    nc.vector.tensor_tensor(b[:], beta, amu[:], op=AO.subtract)
    return a, b


def _allgather_sum(nc, sm, dram, src_ap, P, W, tag):
    """AllGather [P, W] across 8 cores, then local reduce -> [P, W] sums."""
    ag_in = dram.tile([P, W], F32, tag=tag + "i")
    ag_out = dram.tile([N_CORES, P, W], F32, tag=tag + "o")
    nc.gpsimd.dma_start(out=ag_in[:], in_=src_ap)
    nc.gpsimd.collective_compute(
        "AllGather", AO.bypass, replica_groups=[list(range(N_CORES))],
        ins=[ag_in.opt()], outs=[ag_out.opt()])
    g8 = sm.tile([P, W, N_CORES], F32, tag=tag + "g")
    nc.sync.dma_start(out=g8, in_=ag_out[:].rearrange("r p w -> p w r"))
    sg = sm.tile([P, W], F32, tag=tag + "s")
    nc.vector.tensor_reduce(sg[:], g8[:], axis=AX.X, op=AO.add)
    return sg


def _border_memset(nc, t):
    """Zero only the 1-px border of padded [128, nimg, 18, 18] tiles."""
    nc.vector.memset(t[:, :, 0:18:17, :].bitcast(U32), 0)
    nc.vector.memset(t[:, :, 1:17, 0:18:17].bitcast(U32), 0)


# ------------------------------------------------------------- device build

def build(debug=False):
    nc = bacc.Bacc("TRN2", target_bir_lowering=False, debug=False,
                   num_devices=N_CORES)
    din = {}

    def dd(name, shape, dtype=F32R):
        din[name] = nc.dram_tensor(name, list(shape), dtype,
                                   kind="ExternalInput")

    dd("xcol", [54, 32 * 1024])
    dd("w1t2", [54, 128])
    dd("d2", [128, 9 * 128])
    dd("d3", [128, 9 * 128])
    dd("d4", [2, 128, 9 * 128])
    dd("w22t", [128, 128])
    dd("w32t", [128, 256])
    dd("w42t", [2, 128, 256], BF16)
    dd("wfct", [2, 128, 2560], BF16)
    dd("gb", [128, 10], F32)
    dd("gb2", [128, 4], F32)
    dd("foldf", [128, 64], F32)
    dd("dwv4", [128, 2, 9], F32)
    dd("bcastb", [64, 128], F32)
    out_d = nc.dram_tensor("out", [10, B_CORE], F32, kind="ExternalOutput")

    dbg = {}
    if debug:
        for name, shape, dt in [
                ("c1", [128, 32, 18, 18], F32), ("sg1", [64, 2], F32),
                ("ab1", [128, 2], F32), ("h1", [128, 32, 18, 18], F32),
                ("pk2", [128, 2], F32), ("sg2", [64, 2], F32),
                ("ab2", [128, 2], F32),
                ("c2", [128, 32, 18, 18], F32), ("h2", [128, 32, 18, 18], F32),
                ("c3", [2, 128, 16384], BF16),
                ("c4", [2, 128, 16384], BF16), ("h4", [2, 128, 16384], BF16)]:
            dbg[name] = nc.dram_tensor("dbg_" + name, shape, dt,
                                       kind="ExternalOutput")

    with tile.TileContext(nc) as tc:
        with tc.tile_pool(name="wts", bufs=1) as wts, \
             tc.tile_pool(name="sb", bufs=1) as sb, \
             tc.tile_pool(name="sm", bufs=1) as sm, \
             tc.tile_pool(name="xin", bufs=2) as xin, \
             tc.tile_pool(name="cho", bufs=3) as cho, \
             tc.tile_pool(name="ps", bufs=2, space="PSUM") as ps, \
             tc.tile_pool(name="psA", bufs=2, space="PSUM") as psA, \
             tc.tile_pool(name="dram", bufs=1, space="DRAM") as dram:
            _body(nc, tc, wts, sb, sm, xin, cho, ps, psA, dram,
                  din, out_d, dbg)
    nc.compile()
    return nc


def _body(nc, tc, wts, sb, sm, xin, cho, ps, psA, dram,
          din, out_d, dbg):
    # ---------- all weights upfront (HWDGE is idle early; SBUF affordable)
    def wload(name, shape, dtype=F32R):
        t = wts.tile(list(shape), dtype, tag=name)
        nc.sync.dma_start(out=t, in_=din[name].ap())
        return t

    w1t2 = wload("w1t2", [54, 128])
    d2 = wload("d2", [128, 9, 128])
    d3 = wload("d3", [128, 9, 128])
    d4 = wts.tile([128, 2, 9, 128], F32R, tag="d4")
    for g in range(2):
        nc.sync.dma_start(out=d4[:, g], in_=din["d4"].ap()[g])
    w22t = wload("w22t", [128, 128])
    w32t = wload("w32t", [128, 256])
    w42t = wts.tile([128, 2, 256], BF16, tag="w42t")
    for g in range(2):
        nc.sync.dma_start(out=w42t[:, g], in_=din["w42t"].ap()[g])
    wfct = wts.tile([128, 2, 2560], BF16, tag="wfct")
    for g in range(2):
        nc.sync.dma_start(out=wfct[:, g], in_=din["wfct"].ap()[g])
    gb = wload("gb", [128, 10], F32)
    gb2 = wload("gb2", [128, 4], F32)
    foldf = wload("foldf", [128, 64], F32)
    dwv4 = wload("dwv4", [128, 2, 9], F32)
    bcastb = wload("bcastb", [64, 128], F32)

    sqa = sm.tile([128, 512], F32, tag="sqa")   # ACT Square dump
    sqd = sm.tile([128, 512], F32, tag="sqd")   # DVE TTR dump
    c127 = wts.tile([128, 512], F32, tag="c127")
    nc.vector.memset(c127[:], 127.0)

    def bn_fold_gather(pk, gamma64, beta64, tag):
        """pk [128,2]=(S1,S2) per (bh,ch) -> ab128 [128,2] applied coeffs."""
        pm = ps.tile([128, 512], F32, tag="pu")
        pf = pm[0:64, 0:2]
        nc.tensor.matmul(pf, foldf[:], pk[:], start=True, stop=True)
        sA = sm.tile([64, 2], F32, tag=tag + "sA")
        nc.vector.tensor_copy(sA[:], pf)
        sg = _allgather_sum(nc, sm, dram, sA[:], 64, 2, tag + "ag")
        if dbg and tag == "bn1":
            nc.sync.dma_start(out=dbg["sg1"].ap(), in_=sg[:])
        if dbg and tag == "bn2":
            nc.sync.dma_start(out=dbg["pk2"].ap(), in_=pk[:])
            nc.sync.dma_start(out=dbg["sg2"].ap(), in_=sg[:])
        a64, b64 = _emit_ab(nc, sm, sg[:, 0:1], sg[:, 1:2], 64, 1, 131072,
                            gamma64, beta64, tag)
        ab2 = sm.tile([64, 2], F32, tag=tag + "ab2")
        nc.vector.tensor_copy(ab2[:, 0:1], a64[:])
        nc.vector.tensor_copy(ab2[:, 1:2], b64[:])
        pm2 = ps.tile([128, 512], F32, tag="pu")
        pb = pm2[:, 0:2]
        nc.tensor.matmul(pb, bcastb[:], ab2[:], start=True, stop=True)
        ab128 = sm.tile([128, 2], F32, tag=tag + "abb")
        nc.vector.tensor_copy(ab128[:], pb)
        if dbg and tag == "bn1":
            nc.sync.dma_start(out=dbg["ab1"].ap(), in_=ab128[:])
        if dbg and tag == "bn2":
            nc.sync.dma_start(out=dbg["ab2"].ap(), in_=ab128[:])
        return ab128

    # ---------- stage A: conv1 (paired K=54) + 4-phase maxpool -> c1 padded
    c1 = sb.tile([128, 32, 18, 18], F32R, tag="chainA")
    _border_memset(nc, c1)
    c2 = sb.tile([128, 32, 18, 18], F32R, tag="chainB")
    _border_memset(nc, c2)
    h3c = []
    for g in range(2):
        for s in range(2):
            t = sb.tile([128, 2, 18, 18], F32R, tag=f"h3c{g}{s}")
            _border_memset(nc, t)
            h3c.append(t)

    s1a = sm.tile([128, 32], F32, tag="s1a")
    s2a = sm.tile([128, 16], F32, tag="s2a")
    for it in range(16):
        bl = it * 2
        xc = xin.tile([54, 2048], F32R, tag="xc")
        nc.sync.dma_start(
            out=xc, in_=din["xcol"].ap()[:, bl * 1024:(bl + 2) * 1024])
        xv = xc[:].rearrange("k (q y a x b) -> k a b q y x",
                             q=2, y=16, a=2, x=16)
        pA0 = psA.tile([128, 2, 512], F32, tag="pA")
        pA1 = psA.tile([128, 2, 512], F32, tag="pA")
        for i, (dy, dx) in enumerate(((0, 0), (0, 1), (1, 0), (1, 1))):
            dstp = pA0[:, i] if i < 2 else pA1[:, i - 2]
            nc.tensor.matmul(
                dstp.rearrange("p (q y x) -> p q y x", q=2, y=16),
                w1t2[:], xv[:, dy, dx], start=True, stop=True)
        u0 = xin.tile([128, 512], F32, tag="u0")
        nc.scalar.copy(u0[:], pA0[:, 0])
        mA = xin.tile([128, 512], F32, tag="mA")
        nc.vector.tensor_tensor(mA[:], pA0[:, 1], u0[:], op=AO.max)
        mB = xin.tile([128, 512], F32, tag="mB")
        nc.vector.tensor_tensor(mB[:], pA1[:, 0], mA[:], op=AO.max)
        for q in range(2):
            nc.vector.scalar_tensor_tensor(
                c1[:, bl + q, 1:17, 1:17],
                pA1[:, 1, q * 256:(q + 1) * 256]
                .rearrange("p (y x) -> p y x", y=16),
                -128.0,
                mB[:, q * 256:(q + 1) * 256]
                .rearrange("p (y x) -> p y x", y=16),
                op0=AO.max, op1=AO.max,
                accum_out=s1a[:, bl + q:bl + q + 1])
        nc.scalar.activation(
            sqa[:].rearrange("p (a y x) -> p a y x", a=2, y=16),
            c1[:, bl:bl + 2, 1:17, 1:17], AF.Square,
            accum_out=s2a[:, it:it + 1])
    if dbg:
        nc.sync.dma_start(out=dbg["c1"].ap(), in_=c1[:].bitcast(F32))

    # ---------- BN1 (fold bh, gather, AP2) + apply in place
    pk1 = sm.tile([128, 2], F32, tag="pk1")
    nc.vector.tensor_reduce(pk1[:, 0:1], s1a[:], axis=AX.X, op=AO.add)
    nc.vector.tensor_reduce(pk1[:, 1:2], s2a[:], axis=AX.X, op=AO.add)
    ab1 = bn_fold_gather(pk1, gb[0:64, 0:1], gb[0:64, 1:2], "bn1")
    for k in range(8):
        iv = c1[:, 4 * k:4 * k + 4, 1:17, 1:17]
        nc.scalar.activation(iv, iv, AF.Relu, bias=ab1[:, 1:2],
                             scale=ab1[:, 0:1])
    h1 = c1
    if dbg:
        nc.sync.dma_start(out=dbg["h1"].ap(), in_=c1[:].bitcast(F32))

    # ---------- block2: dw2 + 1x1(64->64) -> c2 padded ; BN2 in place
    s1b = sm.tile([128, 32], F32, tag="s1b")
    s2b = sm.tile([128, 16], F32, tag="s2b")
    for b0 in range(0, 32, 2):
        p = _emit_dw(nc, ps, h1[:, b0:b0 + 2], d2, 2, "pdw")
        t2 = cho.tile([128, 2, 16, 16], F32R, tag="tdwA")
        nc.vector.tensor_copy(t2[:], p[:])
        ci = b0 // 2
        pu = ps.tile([128, 512], F32, tag="pu")
        nc.tensor.matmul(
            pu[:], w22t[:], t2[:].rearrange("p a b c -> p (a b c)"),
            start=True, stop=True)
        puv = pu[:].rearrange("p (a b c) -> p a b c", a=2, b=16)
        c127v = c127[:, 0:256].rearrange("p (y x) -> p y x", y=16)
        for si in range(2):
            nc.vector.scalar_tensor_tensor(
                c2[:, b0 + si, 1:17, 1:17], puv[:, si], -128.0, c127v,
                op0=AO.max, op1=AO.min,
                accum_out=s1b[:, 2 * ci + si:2 * ci + si + 1])
        iv = c2[:, b0:b0 + 2, 1:17, 1:17]
        nc.scalar.activation(sqa[:].rearrange("p (a b c) -> p a b c",
                                              a=2, b=16),
                             iv, AF.Square, accum_out=s2b[:, ci:ci + 1])
    if dbg:
        nc.sync.dma_start(out=dbg["c2"].ap(), in_=c2[:].bitcast(F32))

    pk2 = sm.tile([128, 2], F32, tag="pk2")
    nc.vector.tensor_reduce(pk2[:, 0:1], s1b[:], axis=AX.X, op=AO.add)
    nc.vector.tensor_reduce(pk2[:, 1:2], s2b[:], axis=AX.X, op=AO.add)
    ab2c = bn_fold_gather(pk2, gb[0:64, 2:3], gb[0:64, 3:4], "bn2")
    for k in range(8):
        iv = c2[:, 4 * k:4 * k + 4, 1:17, 1:17]
        nc.scalar.activation(iv, iv, AF.Relu, bias=ab2c[:, 1:2],
                             scale=ab2c[:, 0:1])
    h2 = c2
    if dbg:
        nc.sync.dma_start(out=dbg["h2"].ap(), in_=c2[:].bitcast(F32))

    # ---------- block3: dw3 + 1x1(64->256) -> c3 DRAM bf16 (both groups)
    c3_dram = dram.tile([2, 128, 16384], BF16, tag="c3d")
    s13 = sm.tile([128, 2, 32], F32, tag="s13")
    s23 = sm.tile([128, 2, 32], F32, tag="s23")
    for b0 in range(0, 32, 2):
        p = _emit_dw(nc, ps, h2[:, b0:b0 + 2], d3, 2, "pdw")
        t3 = cho.tile([128, 2, 16, 16], F32R, tag="tdwA")
        nc.scalar.copy(t3[:], p[:])
        ci = b0 // 2
        for bh in range(2):
            b_abs = bh * 32 + b0
            for g in range(2):
                pu = ps.tile([128, 512], F32, tag="pu")
                nc.tensor.matmul(
                    pu[:], w32t[bh * 64:(bh + 1) * 64,
                                g * 128:(g + 1) * 128],
                    t3[bh * 64:(bh + 1) * 64]
                    .rearrange("p a b c -> p (a b c)"),
                    start=True, stop=True)
                cc = cho.tile([128, 512], BF16, tag="ccs")
                kcol = bh * 16 + ci
                nc.vector.scalar_tensor_tensor(
                    cc[:], pu[:], -128.0, c127[:], op0=AO.max, op1=AO.min,
                    accum_out=s13[:, g, kcol:kcol + 1])
                if bh == 1 and g == 1:
                    nc.vector.scalar_tensor_tensor(
                        sqd[:], cc[:], 1.0, cc[:],
                        op0=AO.mult, op1=AO.mult,
                        accum_out=s23[:, g, kcol:kcol + 1])
                else:
                    nc.scalar.activation(
                        sqa[:], cc[:], AF.Square,
                        accum_out=s23[:, g, kcol:kcol + 1])
                nc.sync.dma_start(
                    out=c3_dram[g, :, b_abs * 256:(b_abs + 2) * 256],
                    in_=cc[:])
    if dbg:
        for g in range(2):
            nc.sync.dma_start(out=dbg["c3"].ap()[g], in_=c3_dram[g])

    # ---------- BN3: pack both groups (S1g0,S1g1,S2g0,S2g1), one gather
    pk3 = sm.tile([128, 4], F32, tag="pk3")
    for g in range(2):
        nc.vector.tensor_reduce(pk3[:, g:g + 1], s13[:, g],
                                axis=AX.X, op=AO.add)
        nc.vector.tensor_reduce(pk3[:, 2 + g:3 + g], s23[:, g],
                                axis=AX.X, op=AO.add)
    sg3 = _allgather_sum(nc, sm, dram, pk3[:], 128, 4, "bn3ag")
    a3, b3 = _emit_ab(nc, sm, sg3[:, 0:2], sg3[:, 2:4], 128, 2, 131072,
                      gb[:, 4:6], gb[:, 6:8], "bn3")

    # ---------- block4: stream c3 bf16, BN3 on the fly, dw4, 1x1 -> c4 bf16
    c4_g0 = sb.tile([128, 64, 16, 16], BF16, tag="chainA")
    c4_g1 = sb.tile([128, 64, 16, 16], BF16, tag="chainB")
    c4 = [c4_g0, c4_g1]
    s14 = sm.tile([128, 2, 32], F32, tag="s14")
    s24 = sm.tile([128, 2, 32], F32, tag="s24")
    for b0 in range(0, 64, 2):
        ci = b0 // 2
        t4 = []
        for g in range(2):
            c3c = cho.tile([128, 512], BF16, tag="c3c")
            nc.sync.dma_start(out=c3c,
                              in_=c3_dram[g, :, b0 * 256:(b0 + 2) * 256])
            hp = h3c[g * 2 + (ci % 2)]
            nc.scalar.activation(
                hp[:, :, 1:17, 1:17],
                c3c[:].rearrange("p (a b c) -> p a b c", a=2, b=16),
                AF.Relu, bias=b3[:, g:g + 1], scale=a3[:, g:g + 1])
            p = _emit_dw(nc, ps, hp[:], d4[:, g], 2, "pdw",
                         taps=[4, 0, 1, 2, 3, 5, 6])
            tg = cho.tile([128, 2, 16, 16], BF16, tag=f"tdw{'AB'[g]}")
            nc.scalar.copy(tg[:], p[:])
            for t in (7, 8):
                dy, dx = t // 3, t % 3
                for si in range(2):
                    nc.vector.scalar_tensor_tensor(
                        tg[:, si], hp[:, si, dy:dy + 16, dx:dx + 16],
                        dwv4[:, g, t:t + 1], tg[:, si],
                        op0=AO.mult, op1=AO.add)
            t4.append(tg)
        for mg in range(2):
            pu = ps.tile([128, 512], F32, tag="pu")
            for kg in range(2):
                nc.tensor.matmul(
                    pu[:], w42t[:, kg, mg * 128:(mg + 1) * 128],
                    t4[kg][:].rearrange("p a b c -> p (a b c)"),
                    start=(kg == 0), stop=(kg == 1))
            dst = c4[mg][:, b0:b0 + 2].rearrange("p a b c -> p (a b c)")
            nc.vector.scalar_tensor_tensor(
                dst, pu[:], -128.0, c127[:], op0=AO.max, op1=AO.min,
                accum_out=s14[:, mg, ci:ci + 1])
            nc.scalar.activation(sqa[:], dst, AF.Square,
                                 accum_out=s24[:, mg, ci:ci + 1])
    if dbg:
        for g in range(2):
            nc.gpsimd.dma_start(
                out=dbg["c4"].ap()[g],
                in_=c4[g][:].rearrange("p a b c -> p (a b c)"))

    # ---------- BN4: one gather; apply per group, overlapped with FC
    pk4 = sm.tile([128, 4], F32, tag="pk4")
    for g in range(2):
        nc.vector.tensor_reduce(pk4[:, g:g + 1], s14[:, g],
                                axis=AX.X, op=AO.add)
        nc.vector.tensor_reduce(pk4[:, 2 + g:3 + g], s24[:, g],
                                axis=AX.X, op=AO.add)
    sg4 = _allgather_sum(nc, sm, dram, pk4[:], 128, 4, "bn4ag")
    a4, b4 = _emit_ab(nc, sm, sg4[:, 0:2], sg4[:, 2:4], 128, 2, 131072,
                      gb2[:, 0:2], gb2[:, 2:4], "bn4")

    # ---------- FC (bf16): 512 accumulating matmuls K=128, M=10, N=64
    pfm = ps.tile([128, 512], F32, tag="pu")
    pf = pfm[0:10, 0:64]
    n_mm = 0
    for kg in range(2):
        for sl in range(4):
            iv = c4[kg][:, sl * 16:(sl + 1) * 16]
            if kg == 0:
                nc.vector.tensor_scalar(iv, iv, a4[:, 0:1], b4[:, 0:1],
                                        op0=AO.mult, op1=AO.add)
                nc.vector.tensor_scalar(iv, iv, 0.0, None, op0=AO.max)
            else:
                nc.scalar.activation(iv, iv, AF.Relu, bias=b4[:, 1:2],
                                     scale=a4[:, 1:2])
        h4v = c4[kg][:].rearrange("p b y x -> p b (y x)")
        if dbg:
            nc.gpsimd.dma_start(
                out=dbg["h4"].ap()[kg],
                in_=c4[kg][:].rearrange("p a b c -> p (a b c)"))
        for pix in range(256):
            n_mm += 1
            nc.tensor.matmul(pf,
                             wfct[:, kg, pix * 10:(pix + 1) * 10],
                             h4v[:, :, pix],
                             start=(n_mm == 1), stop=(n_mm == 512))
    of = sm.tile([10, 64], F32, tag="of")
    nc.vector.tensor_copy(of[:], pf)
    nc.sync.dma_start(out=out_d.ap(), in_=of[:])


# ------------------------------------------------------------------ kernel

def _prep_inputs(x, w1, w21, w22, w31, w32, w41, w42,
                 g1, b1, g2, b2, g3, b3, g4, b4, wfc):
    (xcol2, w1t2, d2, d3, d4, w22t, w32t, w42t, wfct, foldf,
     bcastb, dwv4) = _host_prep(x, w1, w21, w31, w41, w22, w32, w42, wfc)
    f32 = lambda v: np.asarray(v, np.float32)
    g1, b1, g2, b2 = f32(g1), f32(b1), f32(g2), f32(b2)
    g3, b3, g4, b4 = f32(g3), f32(b3), f32(g4), f32(b4)
    gb = np.zeros((128, 10), np.float32)
    gb[:, 0] = np.tile(g1, 2); gb[:, 1] = np.tile(b1, 2)
    gb[:, 2] = np.tile(g2, 2); gb[:, 3] = np.tile(b2, 2)
    gb[:, 4] = g3[:128]; gb[:, 5] = g3[128:]
    gb[:, 6] = b3[:128]; gb[:, 7] = b3[128:]
    gb2 = np.zeros((128, 4), np.float32)
    gb2[:, 0] = g4[:128]; gb2[:, 1] = g4[128:]
    gb2[:, 2] = b4[:128]; gb2[:, 3] = b4[128:]
    in_maps = []
    for c in range(N_CORES):
        in_maps.append({
            "xcol": xcol2[c], "w1t2": w1t2, "d2": d2, "d3": d3, "d4": d4,
            "w22t": w22t, "w32t": w32t, "w42t": w42t, "wfct": wfct,
            "gb": gb, "gb2": gb2, "foldf": foldf, "bcastb": bcastb,
            "dwv4": dwv4,
        })
    return in_maps


def kernel(x, w1, w21, w22, w31, w32, w41, w42,
           g1, b1, g2, b2, g3, b3, g4, b4, wfc, bfc):
    debug = bool(int(os.environ.get("BCK_DEBUG", "0")))
    key = ("nc", debug)
    if key not in _CACHE:
        _CACHE[key] = build(debug=debug)
    nc = _CACHE[key]
    in_maps = _prep_inputs(x, w1, w21, w22, w31, w32, w41, w42,
                           g1, b1, g2, b2, g3, b3, g4, b4, wfc)
    res = bass_utils.run_bass_kernel_spmd(
        nc, in_maps, core_ids=list(range(N_CORES)))
    kernel.last_results = res
    outs = [res.results[c]["out"] for c in range(N_CORES)]
    full = np.concatenate([o.T for o in outs], axis=0)  # [512, 10]
    return (full + np.asarray(bfc, np.float32)[None, :]).astype(np.float32)
